# revision 37
# baseline (speedup 1.0000x reference)
"""Trainium2 Bass kernel for a GAT block (GATConv + LN + FFN + LN).

Self-contained: builds per-core shards on the host, compiles one SPMD Bass
program, runs it on 8 NeuronCores via run_bass_kernel_spmd, and reassembles
the full [50000, 128] output.

Per-core scheme (core c of 8; nodes assigned to (core, block) bins by
ascending in-degree, snaked across cores so every block index has
near-equal degree on all 8 cores — minimizes the shared edge-padding
profile and makes the last blocks cheap to drain):
  Phase A: for all 50304 (padded) rows compute [h | a_src | a_dst] =
           x @ [W | W@Asrc | W@Adst] on PE (x fed as fp8); store 512-byte
           node rows [h(128) | a_src(8) | a_dst(8) | pad] bf16 to
           core-local DRAM (512B rows keep every DMA descriptor on the
           full-rate >=512B path).
  Phase B: edges with dst owned by the core (incl self-loops), grouped by
           128-node dst block, split lo/hi on the 32K int16 gather-index
           limit, padded per (block, stream) to 128-edge granules with a
           shared max-over-cores profile so all cores run one program.
           Per 4096-edge chunk: gather#1 512B src rows (h + a_src);
           gather#2 32B dst a_dst slivers (raw-constructed descriptor, at
           the 7ns/descriptor floor); the scatter matrix S arrives as a
           host-built fp8 one-hot stream read sequentially at full bus
           rate; eL = a_src + a_dst; p = exp(leaky_relu(eL)) written both
           compact and head-expanded (one Act op each); msg = h * pexp
           in-place over h (DVE 2x); per granule:
           psum[block] += matmul(lhsT=S_fp8, rhs=[msg | p]).
  Phase C: g = agg/denom; u = LN(x + g); ff = relu(u@W1 + b1)@W2 + b2;
           z = LN(u + ff). rstd via exp(-0.5*ln(var+eps)) keeps every
           activation in one pre-placed table set (no LoadActFuncSet
           churn).
"""
import numpy as np
import ml_dtypes

N = 50000
NCORES = 8
OWN = 6272             # nodes per core (49 tiles of 128)
ZPAD = 128             # zero rows after own block (pad-edge target)
NPG = OWN * NCORES     # globally padded node count (50176)
NP2 = NPG + ZPAD       # per-core row count (50304)
BLK = 128
NBLK = OWN // BLK      # 49
GR = 128               # edges per granule
CHUNK = 4096
GPC = CHUNK // GR      # 32
LO_LIM = 1 << 15
H, F, D = 8, 16, 128
ROW = 256              # bf16 cols per node row (512 bytes)
C_AS = 128             # a_src col
C_AD = 136             # a_dst col
LN_EPS = 1e-5

bf16 = ml_dtypes.bfloat16
f8 = ml_dtypes.float8_e4m3fn


def _wrapc(idx):
    """Per-chunk 16-wrap: [CHUNK] int -> [128, CHUNK//16] int16."""
    w = idx.reshape(CHUNK // 16, 16).T.astype(np.int16)
    return np.tile(w, (8, 1))


def _bfr(x):
    return np.ascontiguousarray(x, dtype=np.float32).astype(bf16)


def _build_host_data(inputs):
    x = np.asarray(inputs["x"], np.float32)
    W = np.asarray(inputs["W_gat"], np.float32)
    att_src = np.asarray(inputs["att_src"], np.float32)
    att_dst = np.asarray(inputs["att_dst"], np.float32)
    ei = np.asarray(inputs["edge_index"])

    src = ei[0].astype(np.int64)
    dst = ei[1].astype(np.int64)
    loops = np.arange(N, dtype=np.int64)
    src = np.concatenate([src, loops])
    dst = np.concatenate([dst, loops])

    # Degree-sorted, core-balanced node assignment: rank nodes by in-degree
    # (desc), fill blocks in order (heavy blocks first, light/pad last for a
    # fast drain), snake ranks across cores so each block index has
    # near-equal degree on every core (minimizes the shared max-profile).
    deg = np.zeros(NPG, dtype=np.int64)
    deg[:N] = np.bincount(dst, minlength=N)[:N]  # incl self-loops
    order = np.argsort(deg, kind="stable")
    i = np.arange(NPG)
    row_r = (i % (BLK * NCORES)) // NCORES
    kk = i % NCORES
    core_of_rank = np.where(row_r % 2 == 0, kk, NCORES - 1 - kk)
    node2core = np.empty(NPG, dtype=np.int64)
    node2dl = np.empty(NPG, dtype=np.int64)
    own_nodes = [np.empty(OWN, dtype=np.int64) for _ in range(NCORES)]
    node2core[order] = core_of_rank
    for c in range(NCORES):
        sel = order[core_of_rank == c]          # this core's nodes, rank order
        node2dl[sel] = np.arange(OWN)
        own_nodes[c] = sel

    # per (core, block, stream) counts on row-mapped gather indices
    counts = np.zeros((NCORES, NBLK, 2), dtype=np.int64)
    core_edges = []
    for c in range(NCORES):
        others = np.nonzero(node2core != c)[0]
        rowmap = np.empty(NPG, dtype=np.int64)
        rowmap[others] = OWN + ZPAD + np.arange(others.shape[0])
        rowmap[own_nodes[c]] = np.arange(OWN)
        m = node2core[dst] == c
        s_g = rowmap[src[m]]
        d_l = node2dl[dst[m]]
        blk = d_l // BLK
        lo = s_g < LO_LIM
        core_edges.append((s_g, d_l, blk, lo, others))
        for b in range(NBLK):
            mb = blk == b
            counts[c, b, 0] = np.sum(mb & lo)
            counts[c, b, 1] = np.sum(mb & ~lo)

    g_prof = np.ceil(counts.max(axis=0) / GR).astype(np.int64)   # [NBLK, 2]
    L = [int(g_prof[:, s].sum()) * GR for s in range(2)]
    for s in range(2):
        pad = (-L[s]) % CHUNK
        g_prof[NBLK - 1, s] += pad // GR
        L[s] += pad
    L_LO, L_HI = L

    per_core = []
    for c in range(NCORES):
        s_g, d_l, blk, lo, _others = core_edges[c]
        streams = []
        for sidx in range(2):
            mm = lo if sidx == 0 else ~lo
            Ls = L[sidx]
            gidx = np.zeros(Ls, dtype=np.int64)
            aidx = np.full(Ls, OWN, dtype=np.int64)   # pads -> zero row
            dl = np.full(Ls, -1, dtype=np.int64)      # pads -> no S entry
            pos = 0
            for b in range(NBLK):
                mb = (blk == b) & mm
                k = int(np.sum(mb))
                cap = int(g_prof[b, sidx]) * GR
                gidx[pos:pos + k] = s_g[mb] - (0 if sidx == 0 else LO_LIM)
                aidx[pos:pos + k] = d_l[mb]
                dl[pos:pos + k] = d_l[mb] % BLK
                pos += cap
            # pack [gidx | aidx] wrapped per chunk: [128, nch*512] i16
            nch = Ls // CHUNK
            pk = np.zeros((128, nch * 512), dtype=np.int16)
            for k in range(nch):
                pk[:, k * 512:k * 512 + 256] = _wrapc(gidx[k * CHUNK:(k + 1) * CHUNK])
                pk[:, k * 512 + 256:(k + 1) * 512] = _wrapc(aidx[k * CHUNK:(k + 1) * CHUNK])
            # S stream: per slot a 128-byte fp8 onehot(dl) row, laid out
            # [128 partitions(edge%128), nch*GPC*128 bytes] -> i16 view
            SB = np.zeros((Ls, 128), dtype=np.uint8)
            real = dl >= 0
            SB[np.nonzero(real)[0], dl[real]] = 0x38          # fp8 e4m3 1.0
            SB = SB.reshape(nch, GPC, 128, 128).transpose(2, 0, 1, 3)
            SB = np.ascontiguousarray(SB.reshape(128, nch * GPC * 128))
            streams.append({"idx": np.ascontiguousarray(pk),
                            "S": SB.view(np.int16)})
        per_core.append(streams)

    # weights: Wp = [W | W@Asrc | W@Adst]  -> [128, 144]
    Asrc = np.zeros((D, H), np.float32)
    Adst = np.zeros((D, H), np.float32)
    for h in range(H):
        Asrc[h * F:(h + 1) * F, h] = att_src[h]
        Adst[h * F:(h + 1) * F, h] = att_dst[h]
    Wp = _bfr(np.concatenate([W, W @ Asrc, W @ Adst], axis=1))   # [128, 144]
    I128 = _bfr(np.eye(128, dtype=np.float32))

    xp = np.zeros((NPG, D), np.float32)
    xp[:N] = x
    xT_per_core = []
    x_own_per_core = []
    for c in range(NCORES):
        rows = np.zeros((NP2, D), np.float32)
        rows[0:OWN] = xp[own_nodes[c]]
        rows[OWN + ZPAD:] = xp[core_edges[c][4]]
        xT_per_core.append(np.ascontiguousarray(rows.T.astype(f8)))
        x_own_per_core.append(np.ascontiguousarray(xp[own_nodes[c]]))

    host = {
        "own_nodes": own_nodes,
        "g_prof": g_prof, "L_LO": L_LO, "L_HI": L_HI,
        "per_core": per_core, "xT": xT_per_core, "x_own": x_own_per_core,
        "Wp": Wp, "I128": I128,
        "W1": _bfr(np.asarray(inputs["w_ff1"], np.float32)),
        "W2": _bfr(np.asarray(inputs["w_ff2"], np.float32)),
        "b1col": np.ascontiguousarray(
            np.asarray(inputs["b_ff1"], np.float32).reshape(2, 128).T),
    }
    host["bias_gat"] = np.asarray(inputs["bias_gat"], np.float32)
    host["b_ff2"] = np.asarray(inputs["b_ff2"], np.float32)
    for nm in ("gamma1", "beta1", "gamma2", "beta2"):
        host[nm] = np.asarray(inputs[nm], np.float32)
    host["triv_gb1"] = bool(np.all(host["gamma1"] == 1) and np.all(host["beta1"] == 0))
    host["triv_gb2"] = bool(np.all(host["gamma2"] == 1) and np.all(host["beta2"] == 0))
    host["triv_bgat"] = bool(np.all(host["bias_gat"] == 0))
    host["triv_bff2"] = bool(np.all(host["b_ff2"] == 0))
    return host


def _dma_gather_raw(eng, out_ap, in_ap, idxs_ap, num_idxs, elem_size,
                    elem_step):
    """dma_gather with elem_size below the 256-byte API floor (the floor is a
    transpose-mode restriction; plain row gathers take any size whose row
    pitch is a 256B multiple)."""
    import concourse.mybir as mybir
    from concourse import ap_utils

    assert idxs_ap.dtype == mybir.dt.int16
    assert in_ap.dtype == out_ap.dtype
    assert ap_utils.ap_is_contiguous(out_ap.ap[1:])
    assert ap_utils.ap_is_contiguous(idxs_ap.ap[1:])
    assert in_ap.ap[-1][1] == out_ap.ap[-1][1] == elem_size
    assert in_ap.ap[0][0] == elem_step
    stride_bytes = elem_step * mybir.dt.size(in_ap.dtype)
    assert stride_bytes % 256 == 0
    _in_ap = eng.lower_ap_dma(in_ap, for_custom_bir_dma=True)
    _idxs_ap = eng.lower_ap(idxs_ap)
    _out_ap = eng.lower_ap(out_ap)
    return eng.add_instruction(mybir.InstDMAGatherAnt(
        name=eng.bass.get_next_instruction_name(),
        ins=[*_in_ap, _idxs_ap, eng.lower_val_access(eng.to_reg(num_idxs))],
        outs=[_out_ap],
        transpose=False, num_idxs=num_idxs, elem_size=elem_size,
        stride_bytes_256=stride_bytes // 256, gen_mode=0,
        single_packet=False, queue_num=0,
        sbuf_tokens_per_rank=0, sbuf_free_dim_per_rank=0,
        sbuf_free_dim_pad_per_rank=0, sbuf_byte_offset=0))


def _build_program(host):
    import concourse.bacc as bacc
    import concourse.mybir as mybir
    import concourse.tile as tile

    fp32 = mybir.dt.float32
    bft = mybir.dt.bfloat16
    i16 = mybir.dt.int16
    fp8 = mybir.dt.float8e4
    Alu = mybir.AluOpType
    Act = mybir.ActivationFunctionType

    g_prof = host["g_prof"]
    L_LO, L_HI = host["L_LO"], host["L_HI"]

    nc = bacc.Bacc("TRN2")

    # Pre-place one activation-table load that covers every func we use, so
    # the compile-time fixpoint pass never needs to thrash between sets.
    from concourse.hw_specs import get_activation_tables
    _tabs = list(get_activation_tables(nc.m.arch).items())
    _need = {Act.Exp, Act.Ln, Act.Copy, Act.Relu}
    _set_id = next(i for i, (_n, fns) in enumerate(_tabs) if _need <= fns)

    xT_d = nc.dram_tensor("xT", [128, NP2], fp8, kind="ExternalInput")
    xown_d = nc.dram_tensor("x_own", [OWN, D], fp32, kind="ExternalInput")
    Wp_d = nc.dram_tensor("Wp", [128, 144], bft, kind="ExternalInput")
    I128_d = nc.dram_tensor("I128", [128, 128], bft, kind="ExternalInput")
    W1_d = nc.dram_tensor("W1", [128, 256], bft, kind="ExternalInput")
    W2_d = nc.dram_tensor("W2", [256, 128], bft, kind="ExternalInput")
    b1c_d = nc.dram_tensor("b1col", [128, 2], fp32, kind="ExternalInput")
    gl_d = {}
    if not host["triv_bgat"]:
        gl_d["bgat"] = nc.dram_tensor("bgat_r", [128, 128], fp32, kind="ExternalInput")
    if not host["triv_bff2"]:
        gl_d["bff2"] = nc.dram_tensor("bff2_r", [128, 128], fp32, kind="ExternalInput")
    if not host["triv_gb1"]:
        gl_d["g1"] = nc.dram_tensor("g1_r", [128, 128], fp32, kind="ExternalInput")
        gl_d["b1"] = nc.dram_tensor("b1_r", [128, 128], fp32, kind="ExternalInput")
    if not host["triv_gb2"]:
        gl_d["g2"] = nc.dram_tensor("g2_r", [128, 128], fp32, kind="ExternalInput")
        gl_d["b2"] = nc.dram_tensor("b2_r", [128, 128], fp32, kind="ExternalInput")

    st_d = []
    for sname, Ls in (("lo", L_LO), ("hi", L_HI)):
        st_d.append({
            "idx": nc.dram_tensor(f"idx_{sname}", [128, (Ls // CHUNK) * 512], i16,
                                  kind="ExternalInput"),
            "S": nc.dram_tensor(f"S_{sname}", [128, (Ls // CHUNK) * GPC * 64],
                                i16, kind="ExternalInput"),
            "L": Ls,
        })

    h_d = nc.dram_tensor("h_scratch", [NP2, ROW], bft, kind="Internal")
    z_d = nc.dram_tensor("z", [OWN, D], fp32, kind="ExternalOutput")

    NT2 = NP2 // 128                  # 393 node tiles
    PADT = OWN // 128                 # tile 49 == the zero-pad block
    GT = 3                            # node tiles per psum bank
    SW = 12                           # node tiles per stage flush / x DMA

    with tile.TileContext(nc) as tc:
        nc.scalar.add_instruction(mybir.InstLoadActFuncSet(
            name=nc.get_next_instruction_name(), ins=[], outs=[],
            act_func_set_id=_set_id))
        # ================= consts =================
        cpool = tc.alloc_tile_pool(name="consts", bufs=1)
        Wp_s = cpool.tile([128, 144], bft)
        nc.sync.dma_start(out=Wp_s[:], in_=Wp_d[:])
        I128_s = cpool.tile([128, 128], bft)
        nc.sync.dma_start(out=I128_s[:], in_=I128_d[:])
        W1_s = cpool.tile([128, 256], bft)
        nc.sync.dma_start(out=W1_s[:], in_=W1_d[:])
        W2_s = cpool.tile([256 // 2, 2, 128], bft)
        nc.sync.dma_start(out=W2_s[:],
                          in_=W2_d[:].rearrange("(k h) f -> h k f", k=2))
        b1c_s = cpool.tile([128, 2], fp32)
        nc.sync.dma_start(out=b1c_s[:], in_=b1c_d[:])
        gl_s = {}
        for k, dref in gl_d.items():
            gl_s[k] = cpool.tile([128, 128], fp32, tag=f"gl_{k}")
            nc.sync.dma_start(out=gl_s[k][:], in_=dref[:])
        eps_s = cpool.tile([128, 1], fp32)
        nc.vector.memset(eps_s[:], LN_EPS)
        # two fixed stage buffers (junk cols stay zero)
        stgpool = tc.alloc_tile_pool(name="stg", bufs=1)
        stg = []
        for i in range(2):
            s = stgpool.tile([128, SW, ROW], bft, tag=f"stage{i}")
            nc.vector.memset(s[:], 0.0)
            stg.append(s)

        # ================= phase A =================
        with tc.tile_pool(name="pA", bufs=4) as pA, \
             tc.tile_pool(name="psA", bufs=4, space="PSUM") as psA:
            xt = None
            for tg in range((NT2 + GT - 1) // GT):
                t0 = tg * GT
                ntl = min(GT, NT2 - t0)
                if t0 % SW == 0:
                    nxb = min(SW, NT2 - t0)
                    xt = pA.tile([128, SW * 128], fp8, tag="xt")
                    nc.sync.dma_start(out=xt[:, :nxb * 128],
                                      in_=xT_d[:, t0 * 128:(t0 + nxb) * 128])
                ps = psA.tile([128, GT, 144], fp32, tag="psA")
                for j in range(ntl):
                    jo = (t0 % SW) + j
                    nc.tensor.matmul(ps[:, j, :],
                                     lhsT=xt[:, jo * 128:(jo + 1) * 128],
                                     rhs=Wp_s[:], start=True, stop=True)
                sb = stg[(t0 // SW) % 2]
                j0 = t0 % SW
                if tg % 2 == 0:
                    nc.scalar.activation(out=sb[:, j0:j0 + ntl, 0:144],
                                         in_=ps[:, :ntl, :], func=Act.Copy)
                else:
                    nc.vector.tensor_copy(out=sb[:, j0:j0 + ntl, 0:144],
                                          in_=ps[:, :ntl, :])
                if j0 + ntl == SW or t0 + ntl == NT2:
                    nf = j0 + ntl
                    r0 = (t0 + ntl - nf) * 128
                    nc.sync.dma_start(
                        out=h_d[r0:r0 + nf * 128, :].rearrange(
                            "(j n) d -> n j d", j=nf),
                        in_=sb[:, :nf, :])
        tc.strict_bb_all_engine_barrier()
        stgpool.release()

        # ================= phases B + C =================
        h_lo = h_d[0:LO_LIM, :]
        h_hi = h_d[LO_LIM:NP2, :]
        ad_tab = h_d[:, C_AD:C_AD + 16]   # [NP2, 16] at 512B pitch
        starts = np.zeros((NBLK, 2), dtype=np.int64)
        for s in range(2):
            starts[1:, s] = np.cumsum(g_prof[:-1, s])

        pB = tc.alloc_tile_pool(name="pB", bufs=5)
        pBs = tc.alloc_tile_pool(name="pBsmall", bufs=6)
        psB = tc.alloc_tile_pool(name="psB", bufs=4, space="PSUM")
        pC = tc.alloc_tile_pool(name="pC", bufs=2)
        psC = tc.alloc_tile_pool(name="psC", bufs=1, space="PSUM")

        chunk_tiles = [{}, {}]

        def emit_chunk(s, k, prefetch=True):
            if k in chunk_tiles[s]:
                return chunk_tiles[s][k]
            sd = st_d[s]
            idx = pBs.tile([128, 512], i16, tag="idx")
            nc.sync.dma_start(out=idx[:],
                              in_=sd["idx"][:, k * 512:(k + 1) * 512])
            Sg = pB.tile([128, GPC, 64], i16, tag="Sg")
            nc.sync.dma_start(out=Sg[:],
                              in_=sd["S"][:, k * GPC * 64:(k + 1) * GPC * 64]
                              .rearrange("p (g c) -> p g c", c=64))
            hrow = pB.tile([128, GPC, ROW], bft, tag="h")
            nc.gpsimd.dma_gather(hrow[:], h_lo if s == 0 else h_hi,
                                 idx[:, 0:256], CHUNK, CHUNK, ROW,
                                 single_packet=False)
            meta = pB.tile([128, GPC, 16], bft, tag="m")
            _dma_gather_raw(nc.gpsimd, meta[:], ad_tab, idx[:, 256:512],
                            CHUNK, 16, ROW)
            # eL = a_src[src] + a_dst[dst]
            eL = pBs.tile([128, GPC, 8], bft, tag="eL")
            nc.vector.tensor_tensor(out=eL[:], in0=hrow[:, :, C_AS:C_AS + 8],
                                    in1=meta[:, :, 0:8], op=Alu.add)
            eL2 = pBs.tile([128, GPC, 8], bft, tag="eL2")
            nc.vector.scalar_tensor_tensor(out=eL2[:], in0=eL[:], scalar=0.2,
                                           in1=eL[:], op0=Alu.mult, op1=Alu.max)
            # p over the spent a_src cols; msg in place over h
            nc.scalar.activation(out=hrow[:, :, 128:136], in_=eL2[:],
                                 func=Act.Exp)
            pexp = pB.tile([128, GPC, 128], bft, tag="px")
            nc.scalar.activation(
                out=pexp[:].rearrange("p g (h f) -> p g h f", f=F),
                in_=eL2[:].to_broadcast([128, GPC, 8, F]), func=Act.Exp)
            nc.vector.tensor_tensor(out=hrow[:, :, 0:128],
                                    in0=hrow[:, :, 0:128], in1=pexp[:],
                                    op=Alu.mult)
            res = {"S": Sg, "msgp": hrow}
            chunk_tiles[s][k] = res
            # prefetch one chunk ahead (non-recursive) so its gathers are in
            # the instruction stream before this chunk's consumers
            if prefetch and (k + 1) * CHUNK < sd["L"]:
                emit_chunk(s, k + 1, prefetch=False)
            return res

        for b in range(NBLK):
            ps_blk = psB.tile([128, 136], fp32, tag="blk")
            tot = int(g_prof[b, 0] + g_prof[b, 1])
            done = 0
            for s in range(2):
                for gi in range(int(g_prof[b, s])):
                    gg = int(starts[b, s]) + gi
                    ct = emit_chunk(s, gg // GPC)
                    gl = gg % GPC
                    nc.tensor.matmul(
                        ps_blk[:],
                        lhsT=ct["S"][:, gl, :].bitcast(fp8),
                        rhs=ct["msgp"][:, gl, 0:136],
                        start=(done == 0), stop=(done == tot - 1))
                    done += 1
            rec = pBs.tile([128, 8], fp32, tag="rec")
            nc.vector.reciprocal(out=rec[:], in_=ps_blk[:, 128:136])
            gt = pC.tile([128, 128], fp32, tag="gt")
            nc.vector.tensor_tensor(
                out=gt[:].rearrange("p (h f) -> p h f", f=F),
                in0=ps_blk[:, 0:128].rearrange("p (h f) -> p h f", f=F),
                in1=rec[:].to_broadcast([128, 8, F]), op=Alu.mult)
            if not host["triv_bgat"]:
                nc.vector.tensor_tensor(out=gt[:], in0=gt[:], in1=gl_s["bgat"][:],
                                        op=Alu.add)
            # ---- phase C for tile b ----
            xo = pC.tile([128, 128], fp32, tag="xo")
            nc.sync.dma_start(out=xo[:], in_=xown_d[b * 128:(b + 1) * 128, :])
            t1 = pC.tile([128, 128], fp32, tag="t1")
            nc.vector.tensor_tensor(out=t1[:], in0=xo[:], in1=gt[:], op=Alu.add)

            def layer_norm(tin, g_key, b_key, triv, tagp):
                bst = pBs.tile([128, 6], fp32, tag=f"bst{tagp}")
                nc.vector.bn_stats(out=bst[:], in_=tin[:])
                mv = pBs.tile([128, 2], fp32, tag=f"mv{tagp}")
                nc.vector.bn_aggr(out=mv[:], in_=bst[:])
                # rstd = exp(-0.5 * ln(var + eps)); stays in the exp/ln set
                nc.scalar.activation(out=mv[:, 1:2], in_=mv[:, 1:2],
                                     func=Act.Ln, bias=eps_s[:])
                nc.scalar.activation(out=mv[:, 1:2], in_=mv[:, 1:2],
                                     func=Act.Exp, scale=-0.5)
                o = pC.tile([128, 128], fp32, tag=f"ln{tagp}")
                nc.vector.tensor_scalar(out=o[:], in0=tin[:],
                                        scalar1=mv[:, 0:1], op0=Alu.subtract,
                                        scalar2=mv[:, 1:2], op1=Alu.mult)
                if not triv:
                    nc.vector.tensor_tensor(out=o[:], in0=o[:], in1=gl_s[g_key][:],
                                            op=Alu.mult)
                    nc.vector.tensor_tensor(out=o[:], in0=o[:], in1=gl_s[b_key][:],
                                            op=Alu.add)
                return o

            u = layer_norm(t1, "g1", "b1", host["triv_gb1"], "1")
            u_bf = pC.tile([128, 128], bft, tag="ubf")
            nc.scalar.activation(out=u_bf[:], in_=u[:], func=Act.Copy)
            uT_ps = psC.tile([128, 128], bft, tag="uT")
            nc.tensor.transpose(uT_ps[:], in_=u_bf[:], identity=I128_s[:])
            uT = pC.tile([128, 128], bft, tag="uTs")
            nc.scalar.activation(out=uT[:], in_=uT_ps[:], func=Act.Copy)
            f1ps = psC.tile([128, 2, 128], fp32, tag="f1")
            for j in range(2):
                nc.tensor.matmul(f1ps[:, j, :], lhsT=W1_s[:, j * 128:(j + 1) * 128],
                                 rhs=uT[:], start=True, stop=True)
            r1 = pC.tile([128, 2, 128], bft, tag="r1")
            for j in range(2):
                nc.scalar.activation(out=r1[:, j, :], in_=f1ps[:, j, :],
                                     func=Act.Relu, bias=b1c_s[:, j:j + 1])
            zps = psC.tile([128, 128], fp32, tag="zp")
            for j in range(2):
                nc.tensor.matmul(zps[:], lhsT=r1[:, j, :], rhs=W2_s[:, j, :],
                                 start=(j == 0), stop=(j == 1))
            t2 = pC.tile([128, 128], fp32, tag="t2")
            nc.vector.tensor_tensor(out=t2[:], in0=u[:], in1=zps[:], op=Alu.add)
            if not host["triv_bff2"]:
                nc.vector.tensor_tensor(out=t2[:], in0=t2[:], in1=gl_s["bff2"][:],
                                        op=Alu.add)
            zt = layer_norm(t2, "g2", "b2", host["triv_gb2"], "2")
            nc.sync.dma_start(out=z_d[b * 128:(b + 1) * 128, :], in_=zt[:])

        for p in (psC, pC, psB, pBs, pB):
            p.release()
        cpool.release()

    nc.compile()
    return nc


def kernel(**inputs):
    from concourse.bass_utils import run_bass_kernel_spmd
    import os

    host = _build_host_data(inputs)
    nc = _build_program(host)

    in_maps = []
    for c in range(NCORES):
        m = {
            "xT": host["xT"][c],
            "x_own": host["x_own"][c],
            "Wp": host["Wp"], "I128": host["I128"],
            "W1": host["W1"], "W2": host["W2"], "b1col": host["b1col"],
        }
        if not host["triv_bgat"]:
            m["bgat_r"] = np.tile(host["bias_gat"].reshape(1, -1), (128, 1))
        if not host["triv_bff2"]:
            m["bff2_r"] = np.tile(host["b_ff2"].reshape(1, -1), (128, 1))
        if not host["triv_gb1"]:
            m["g1_r"] = np.tile(host["gamma1"].reshape(1, -1), (128, 1))
            m["b1_r"] = np.tile(host["beta1"].reshape(1, -1), (128, 1))
        if not host["triv_gb2"]:
            m["g2_r"] = np.tile(host["gamma2"].reshape(1, -1), (128, 1))
            m["b2_r"] = np.tile(host["beta2"].reshape(1, -1), (128, 1))
        for s, sname in ((0, "lo"), (1, "hi")):
            m[f"idx_{sname}"] = host["per_core"][c][s]["idx"]
            m[f"S_{sname}"] = host["per_core"][c][s]["S"]
        in_maps.append(m)

    trace = bool(int(os.environ.get("GAT_TRACE", "0")))
    res = run_bass_kernel_spmd(nc, in_maps, core_ids=list(range(NCORES)),
                               trace=trace)
    if trace and res.exec_time_ns:
        print(f"HW exec time: {res.exec_time_ns} ns")
    if bool(int(os.environ.get("GAT_TIME", "0"))):
        try:
            from concourse.timeline_sim import TimelineSim
            ts = TimelineSim(nc)
            dur = ts.simulate()
            print(f"HW exec time: {dur:.0f} ns (cost-model timeline estimate)")
        except Exception as e:
            print("timeline sim failed:", e)

    out = np.zeros((N, D), np.float32)
    for c in range(NCORES):
        on = host["own_nodes"][c]
        mreal = on < N
        out[on[mreal]] = res.results[c]["z"][mreal]
    return out


# revision 39
# speedup vs baseline: 1.0479x; 1.0479x over previous
"""Trainium2 Bass kernel for a GAT block (GATConv + LN + FFN + LN).

Self-contained: builds per-core shards on the host, compiles one SPMD Bass
program, runs it on 8 NeuronCores via run_bass_kernel_spmd, and reassembles
the full [50000, 128] output.

Per-core scheme (core c of 8; nodes assigned to (core, block) bins by
ascending in-degree, snaked across cores so every block index has
near-equal degree on all 8 cores — minimizes the shared edge-padding
profile and makes the last blocks cheap to drain):
  Phase A: for all 50304 (padded) rows compute [h | a_src | a_dst] =
           x @ [W | W@Asrc | W@Adst] on PE (x fed as fp8); store 512-byte
           node rows [h(128) | a_src(8) | a_dst(8) | pad] bf16 to
           core-local DRAM (512B rows keep every DMA descriptor on the
           full-rate >=512B path).
  Phase B: edges with dst owned by the core (incl self-loops), grouped by
           128-node dst block, split lo/hi on the 32K int16 gather-index
           limit, padded per (block, stream) to 128-edge granules with a
           shared max-over-cores profile so all cores run one program.
           Per 4096-edge chunk: gather#1 512B src rows (h + a_src);
           gather#2 32B dst a_dst slivers (raw-constructed descriptor, at
           the 7ns/descriptor floor); the scatter matrix S arrives as a
           host-built fp8 one-hot stream read sequentially at full bus
           rate; eL = a_src + a_dst; p = exp(leaky_relu(eL)) written both
           compact and head-expanded (one Act op each); msg = h * pexp
           in-place over h (DVE 2x); per granule:
           psum[block] += matmul(lhsT=S_fp8, rhs=[msg | p]).
  Phase C: g = agg/denom; u = LN(x + g); ff = relu(u@W1 + b1)@W2 + b2;
           z = LN(u + ff). rstd via exp(-0.5*ln(var+eps)) keeps every
           activation in one pre-placed table set (no LoadActFuncSet
           churn).
"""
import numpy as np
import ml_dtypes

N = 50000
NCORES = 8
OWN = 6272             # nodes per core (49 tiles of 128)
ZPAD = 128             # zero rows after own block (pad-edge target)
NPG = OWN * NCORES     # globally padded node count (50176)
NP2 = NPG + ZPAD       # per-core row count (50304)
BLK = 128
NBLK = OWN // BLK      # 49
GR = 128               # edges per granule
CHUNK = 4096
GPC = CHUNK // GR      # 32
LO_LIM = 1 << 15
H, F, D = 8, 16, 128
ROW = 256              # bf16 cols per node row (512 bytes)
C_AS = 128             # a_src col
C_AD = 136             # a_dst col
LN_EPS = 1e-5

bf16 = ml_dtypes.bfloat16
f8 = ml_dtypes.float8_e4m3fn


def _wrapc(idx):
    """Per-chunk 16-wrap: [CHUNK] int -> [128, CHUNK//16] int16."""
    w = idx.reshape(CHUNK // 16, 16).T.astype(np.int16)
    return np.tile(w, (8, 1))


def _bfr(x):
    return np.ascontiguousarray(x, dtype=np.float32).astype(bf16)


def _build_host_data(inputs):
    x = np.asarray(inputs["x"], np.float32)
    W = np.asarray(inputs["W_gat"], np.float32)
    att_src = np.asarray(inputs["att_src"], np.float32)
    att_dst = np.asarray(inputs["att_dst"], np.float32)
    ei = np.asarray(inputs["edge_index"])

    src = ei[0].astype(np.int64)
    dst = ei[1].astype(np.int64)
    loops = np.arange(N, dtype=np.int64)
    src = np.concatenate([src, loops])
    dst = np.concatenate([dst, loops])

    # Degree-sorted, core-balanced node assignment: rank nodes by in-degree
    # (desc), fill blocks in order (heavy blocks first, light/pad last for a
    # fast drain), snake ranks across cores so each block index has
    # near-equal degree on every core (minimizes the shared max-profile).
    deg = np.zeros(NPG, dtype=np.int64)
    deg[:N] = np.bincount(dst, minlength=N)[:N]  # incl self-loops
    order = np.argsort(deg, kind="stable")
    i = np.arange(NPG)
    row_r = (i % (BLK * NCORES)) // NCORES
    kk = i % NCORES
    core_of_rank = np.where(row_r % 2 == 0, kk, NCORES - 1 - kk)
    node2core = np.empty(NPG, dtype=np.int64)
    node2dl = np.empty(NPG, dtype=np.int64)
    own_nodes = [np.empty(OWN, dtype=np.int64) for _ in range(NCORES)]
    node2core[order] = core_of_rank
    for c in range(NCORES):
        sel = order[core_of_rank == c]          # this core's nodes, rank order
        node2dl[sel] = np.arange(OWN)
        own_nodes[c] = sel

    # per (core, block, stream) counts on row-mapped gather indices
    counts = np.zeros((NCORES, NBLK, 2), dtype=np.int64)
    core_edges = []
    for c in range(NCORES):
        others = np.nonzero(node2core != c)[0]
        rowmap = np.empty(NPG, dtype=np.int64)
        rowmap[others] = OWN + ZPAD + np.arange(others.shape[0])
        rowmap[own_nodes[c]] = np.arange(OWN)
        m = node2core[dst] == c
        s_g = rowmap[src[m]]
        d_l = node2dl[dst[m]]
        blk = d_l // BLK
        lo = s_g < LO_LIM
        core_edges.append((s_g, d_l, blk, lo, others))
        for b in range(NBLK):
            mb = blk == b
            counts[c, b, 0] = np.sum(mb & lo)
            counts[c, b, 1] = np.sum(mb & ~lo)

    g_prof = np.ceil(counts.max(axis=0) / GR).astype(np.int64)   # [NBLK, 2]
    L = [int(g_prof[:, s].sum()) * GR for s in range(2)]
    for s in range(2):
        pad = (-L[s]) % CHUNK
        g_prof[NBLK - 1, s] += pad // GR
        L[s] += pad
    L_LO, L_HI = L

    per_core = []
    for c in range(NCORES):
        s_g, d_l, blk, lo, _others = core_edges[c]
        streams = []
        for sidx in range(2):
            mm = lo if sidx == 0 else ~lo
            Ls = L[sidx]
            gidx = np.zeros(Ls, dtype=np.int64)
            aidx = np.full(Ls, OWN, dtype=np.int64)   # pads -> zero row
            dl = np.full(Ls, -1, dtype=np.int64)      # pads -> no S entry
            pos = 0
            for b in range(NBLK):
                mb = (blk == b) & mm
                k = int(np.sum(mb))
                cap = int(g_prof[b, sidx]) * GR
                gidx[pos:pos + k] = s_g[mb] - (0 if sidx == 0 else LO_LIM)
                aidx[pos:pos + k] = d_l[mb]
                dl[pos:pos + k] = d_l[mb] % BLK
                pos += cap
            # per chunk [gidx 256 | aidx 256 | S 2048] i16 -> one DMA each
            nch = Ls // CHUNK
            SB = np.zeros((Ls, 128), dtype=np.uint8)
            real = dl >= 0
            SB[np.nonzero(real)[0], dl[real]] = 0x38          # fp8 e4m3 1.0
            SB = SB.reshape(nch, GPC, 128, 128).transpose(2, 0, 1, 3)
            S16 = np.ascontiguousarray(SB).reshape(
                128, nch, GPC * 128).view(np.int16)
            pk = np.zeros((128, nch * 2560), dtype=np.int16)
            for k in range(nch):
                pk[:, k * 2560:k * 2560 + 256] = _wrapc(gidx[k * CHUNK:(k + 1) * CHUNK])
                pk[:, k * 2560 + 256:k * 2560 + 512] = _wrapc(aidx[k * CHUNK:(k + 1) * CHUNK])
                pk[:, k * 2560 + 512:(k + 1) * 2560] = S16[:, k, :]
            streams.append({"idx": np.ascontiguousarray(pk)})
        per_core.append(streams)

    # weights: Wp = [W | W@Asrc | W@Adst]  -> [128, 144]
    Asrc = np.zeros((D, H), np.float32)
    Adst = np.zeros((D, H), np.float32)
    for h in range(H):
        Asrc[h * F:(h + 1) * F, h] = att_src[h]
        Adst[h * F:(h + 1) * F, h] = att_dst[h]
    Wp = _bfr(np.concatenate([W, W @ Asrc, W @ Adst], axis=1))   # [128, 144]
    I128 = _bfr(np.eye(128, dtype=np.float32))

    xp = np.zeros((NPG, D), np.float32)
    xp[:N] = x
    xT_per_core = []
    x_own_per_core = []
    for c in range(NCORES):
        rows = np.zeros((NP2, D), np.float32)
        rows[0:OWN] = xp[own_nodes[c]]
        rows[OWN + ZPAD:] = xp[core_edges[c][4]]
        xT_per_core.append(np.ascontiguousarray(rows.T.astype(f8)))
        x_own_per_core.append(np.ascontiguousarray(xp[own_nodes[c]]))

    host = {
        "own_nodes": own_nodes,
        "g_prof": g_prof, "L_LO": L_LO, "L_HI": L_HI,
        "per_core": per_core, "xT": xT_per_core, "x_own": x_own_per_core,
        "Wp": Wp, "I128": I128,
        "W1": _bfr(np.asarray(inputs["w_ff1"], np.float32)),
        "W2": _bfr(np.asarray(inputs["w_ff2"], np.float32)),
        "b1col": np.ascontiguousarray(
            np.asarray(inputs["b_ff1"], np.float32).reshape(2, 128).T),
    }
    host["bias_gat"] = np.asarray(inputs["bias_gat"], np.float32)
    host["b_ff2"] = np.asarray(inputs["b_ff2"], np.float32)
    for nm in ("gamma1", "beta1", "gamma2", "beta2"):
        host[nm] = np.asarray(inputs[nm], np.float32)
    host["triv_gb1"] = bool(np.all(host["gamma1"] == 1) and np.all(host["beta1"] == 0))
    host["triv_gb2"] = bool(np.all(host["gamma2"] == 1) and np.all(host["beta2"] == 0))
    host["triv_bgat"] = bool(np.all(host["bias_gat"] == 0))
    host["triv_bff2"] = bool(np.all(host["b_ff2"] == 0))
    return host


def _dma_gather_raw(eng, out_ap, in_ap, idxs_ap, num_idxs, elem_size,
                    elem_step):
    """dma_gather with elem_size below the 256-byte API floor (the floor is a
    transpose-mode restriction; plain row gathers take any size whose row
    pitch is a 256B multiple)."""
    import concourse.mybir as mybir
    from concourse import ap_utils

    assert idxs_ap.dtype == mybir.dt.int16
    assert in_ap.dtype == out_ap.dtype
    assert ap_utils.ap_is_contiguous(out_ap.ap[1:])
    assert ap_utils.ap_is_contiguous(idxs_ap.ap[1:])
    assert in_ap.ap[-1][1] == out_ap.ap[-1][1] == elem_size
    assert in_ap.ap[0][0] == elem_step
    stride_bytes = elem_step * mybir.dt.size(in_ap.dtype)
    assert stride_bytes % 256 == 0
    _in_ap = eng.lower_ap_dma(in_ap, for_custom_bir_dma=True)
    _idxs_ap = eng.lower_ap(idxs_ap)
    _out_ap = eng.lower_ap(out_ap)
    return eng.add_instruction(mybir.InstDMAGatherAnt(
        name=eng.bass.get_next_instruction_name(),
        ins=[*_in_ap, _idxs_ap, eng.lower_val_access(eng.to_reg(num_idxs))],
        outs=[_out_ap],
        transpose=False, num_idxs=num_idxs, elem_size=elem_size,
        stride_bytes_256=stride_bytes // 256, gen_mode=0,
        single_packet=False, queue_num=0,
        sbuf_tokens_per_rank=0, sbuf_free_dim_per_rank=0,
        sbuf_free_dim_pad_per_rank=0, sbuf_byte_offset=0))


def _build_program(host):
    import concourse.bacc as bacc
    import concourse.mybir as mybir
    import concourse.tile as tile

    fp32 = mybir.dt.float32
    bft = mybir.dt.bfloat16
    i16 = mybir.dt.int16
    fp8 = mybir.dt.float8e4
    Alu = mybir.AluOpType
    Act = mybir.ActivationFunctionType

    g_prof = host["g_prof"]
    L_LO, L_HI = host["L_LO"], host["L_HI"]

    nc = bacc.Bacc("TRN2")

    # Pre-place one activation-table load that covers every func we use, so
    # the compile-time fixpoint pass never needs to thrash between sets.
    from concourse.hw_specs import get_activation_tables
    _tabs = list(get_activation_tables(nc.m.arch).items())
    _need = {Act.Exp, Act.Ln, Act.Copy, Act.Relu}
    _set_id = next(i for i, (_n, fns) in enumerate(_tabs) if _need <= fns)

    xT_d = nc.dram_tensor("xT", [128, NP2], fp8, kind="ExternalInput")
    xown_d = nc.dram_tensor("x_own", [OWN, D], fp32, kind="ExternalInput")
    Wp_d = nc.dram_tensor("Wp", [128, 144], bft, kind="ExternalInput")
    I128_d = nc.dram_tensor("I128", [128, 128], bft, kind="ExternalInput")
    W1_d = nc.dram_tensor("W1", [128, 256], bft, kind="ExternalInput")
    W2_d = nc.dram_tensor("W2", [256, 128], bft, kind="ExternalInput")
    b1c_d = nc.dram_tensor("b1col", [128, 2], fp32, kind="ExternalInput")
    gl_d = {}
    if not host["triv_bgat"]:
        gl_d["bgat"] = nc.dram_tensor("bgat_r", [128, 128], fp32, kind="ExternalInput")
    if not host["triv_bff2"]:
        gl_d["bff2"] = nc.dram_tensor("bff2_r", [128, 128], fp32, kind="ExternalInput")
    if not host["triv_gb1"]:
        gl_d["g1"] = nc.dram_tensor("g1_r", [128, 128], fp32, kind="ExternalInput")
        gl_d["b1"] = nc.dram_tensor("b1_r", [128, 128], fp32, kind="ExternalInput")
    if not host["triv_gb2"]:
        gl_d["g2"] = nc.dram_tensor("g2_r", [128, 128], fp32, kind="ExternalInput")
        gl_d["b2"] = nc.dram_tensor("b2_r", [128, 128], fp32, kind="ExternalInput")

    st_d = []
    for sname, Ls in (("lo", L_LO), ("hi", L_HI)):
        st_d.append({
            "idx": nc.dram_tensor(f"idx_{sname}", [128, (Ls // CHUNK) * 2560],
                                  i16, kind="ExternalInput"),
            "L": Ls,
        })

    h_d = nc.dram_tensor("h_scratch", [NP2, ROW], bft, kind="Internal")
    z_d = nc.dram_tensor("z", [OWN, D], fp32, kind="ExternalOutput")

    NT2 = NP2 // 128                  # 393 node tiles
    PADT = OWN // 128                 # tile 49 == the zero-pad block
    GT = 3                            # node tiles per psum bank
    SW = 12                           # node tiles per stage flush / x DMA

    with tile.TileContext(nc) as tc:
        nc.scalar.add_instruction(mybir.InstLoadActFuncSet(
            name=nc.get_next_instruction_name(), ins=[], outs=[],
            act_func_set_id=_set_id))
        # ================= consts =================
        cpool = tc.alloc_tile_pool(name="consts", bufs=1)
        Wp_s = cpool.tile([128, 144], bft)
        nc.sync.dma_start(out=Wp_s[:], in_=Wp_d[:])
        I128_s = cpool.tile([128, 128], bft)
        nc.sync.dma_start(out=I128_s[:], in_=I128_d[:])
        W1_s = cpool.tile([128, 256], bft)
        nc.sync.dma_start(out=W1_s[:], in_=W1_d[:])
        W2_s = cpool.tile([256 // 2, 2, 128], bft)
        nc.sync.dma_start(out=W2_s[:],
                          in_=W2_d[:].rearrange("(k h) f -> h k f", k=2))
        b1c_s = cpool.tile([128, 2], fp32)
        nc.sync.dma_start(out=b1c_s[:], in_=b1c_d[:])
        gl_s = {}
        for k, dref in gl_d.items():
            gl_s[k] = cpool.tile([128, 128], fp32, tag=f"gl_{k}")
            nc.sync.dma_start(out=gl_s[k][:], in_=dref[:])
        eps_s = cpool.tile([128, 1], fp32)
        nc.vector.memset(eps_s[:], LN_EPS)
        # two fixed stage buffers (junk cols stay zero)
        stgpool = tc.alloc_tile_pool(name="stg", bufs=1)
        stg = []
        for i in range(2):
            s = stgpool.tile([128, SW, ROW], bft, tag=f"stage{i}")
            nc.vector.memset(s[:], 0.0)
            stg.append(s)

        # ================= phase A =================
        with tc.tile_pool(name="pA", bufs=4) as pA, \
             tc.tile_pool(name="psA", bufs=4, space="PSUM") as psA:
            xt = None
            for tg in range((NT2 + GT - 1) // GT):
                t0 = tg * GT
                ntl = min(GT, NT2 - t0)
                if t0 % SW == 0:
                    nxb = min(SW, NT2 - t0)
                    xt = pA.tile([128, SW * 128], fp8, tag="xt")
                    nc.sync.dma_start(out=xt[:, :nxb * 128],
                                      in_=xT_d[:, t0 * 128:(t0 + nxb) * 128])
                ps = psA.tile([128, GT, 144], fp32, tag="psA")
                for j in range(ntl):
                    jo = (t0 % SW) + j
                    nc.tensor.matmul(ps[:, j, :],
                                     lhsT=xt[:, jo * 128:(jo + 1) * 128],
                                     rhs=Wp_s[:], start=True, stop=True)
                sb = stg[(t0 // SW) % 2]
                j0 = t0 % SW
                if tg % 2 == 0:
                    nc.scalar.activation(out=sb[:, j0:j0 + ntl, 0:144],
                                         in_=ps[:, :ntl, :], func=Act.Copy)
                else:
                    nc.vector.tensor_copy(out=sb[:, j0:j0 + ntl, 0:144],
                                          in_=ps[:, :ntl, :])
                if j0 + ntl == SW or t0 + ntl == NT2:
                    nf = j0 + ntl
                    r0 = (t0 + ntl - nf) * 128
                    nc.sync.dma_start(
                        out=h_d[r0:r0 + nf * 128, :].rearrange(
                            "(j n) d -> n j d", j=nf),
                        in_=sb[:, :nf, :])
        tc.strict_bb_all_engine_barrier()
        stgpool.release()

        # ================= phases B + C =================
        h_lo = h_d[0:LO_LIM, :]
        h_hi = h_d[LO_LIM:NP2, :]
        ad_tab = h_d[:, C_AD:C_AD + 16]   # [NP2, 16] at 512B pitch
        starts = np.zeros((NBLK, 2), dtype=np.int64)
        for s in range(2):
            starts[1:, s] = np.cumsum(g_prof[:-1, s])

        pB = tc.alloc_tile_pool(name="pB", bufs=6)
        pBs = tc.alloc_tile_pool(name="pBsmall", bufs=6)
        psB = tc.alloc_tile_pool(name="psB", bufs=5, space="PSUM")
        pC = tc.alloc_tile_pool(name="pC", bufs=3)
        psC = tc.alloc_tile_pool(name="psC", bufs=1, space="PSUM")

        chunk_tiles = [{}, {}]

        def emit_chunk(s, k, prefetch=True):
            if k in chunk_tiles[s]:
                return chunk_tiles[s][k]
            sd = st_d[s]
            idx = pB.tile([128, 2560], i16, tag="idx")
            nc.sync.dma_start(out=idx[:],
                              in_=sd["idx"][:, k * 2560:(k + 1) * 2560])
            hrow = pB.tile([128, GPC, ROW], bft, tag="h")
            nc.gpsimd.dma_gather(hrow[:], h_lo if s == 0 else h_hi,
                                 idx[:, 0:256], CHUNK, CHUNK, ROW,
                                 single_packet=False)
            meta = pB.tile([128, GPC, 16], bft, tag="m")
            _dma_gather_raw(nc.gpsimd, meta[:], ad_tab, idx[:, 256:512],
                            CHUNK, 16, ROW)
            # eL = a_src[src] + a_dst[dst]
            eL = pBs.tile([128, GPC, 8], bft, tag="eL")
            nc.vector.tensor_tensor(out=eL[:], in0=hrow[:, :, C_AS:C_AS + 8],
                                    in1=meta[:, :, 0:8], op=Alu.add)
            eL2 = pBs.tile([128, GPC, 8], bft, tag="eL2")
            nc.vector.scalar_tensor_tensor(out=eL2[:], in0=eL[:], scalar=0.2,
                                           in1=eL[:], op0=Alu.mult, op1=Alu.max)
            # p over the spent a_src cols; msg in place over h
            nc.scalar.activation(out=hrow[:, :, 128:136], in_=eL2[:],
                                 func=Act.Exp)
            pexp = pB.tile([128, GPC, 128], bft, tag="px")
            nc.scalar.activation(
                out=pexp[:].rearrange("p g (h f) -> p g h f", f=F),
                in_=eL2[:].to_broadcast([128, GPC, 8, F]), func=Act.Exp)
            nc.vector.tensor_tensor(out=hrow[:, :, 0:128],
                                    in0=hrow[:, :, 0:128], in1=pexp[:],
                                    op=Alu.mult)
            res = {"S": idx, "msgp": hrow}
            chunk_tiles[s][k] = res
            # prefetch one chunk ahead (non-recursive) so its gathers are in
            # the instruction stream before this chunk's consumers
            if prefetch and (k + 1) * CHUNK < sd["L"]:
                emit_chunk(s, k + 1, prefetch=False)
            return res

        for b in range(NBLK):
            ps_blk = psB.tile([128, 136], fp32, tag="blk")
            tot = int(g_prof[b, 0] + g_prof[b, 1])
            done = 0
            for s in range(2):
                for gi in range(int(g_prof[b, s])):
                    gg = int(starts[b, s]) + gi
                    ct = emit_chunk(s, gg // GPC)
                    gl = gg % GPC
                    nc.tensor.matmul(
                        ps_blk[:],
                        lhsT=ct["S"][:, 512 + gl * 64:512 + (gl + 1) * 64]
                        .bitcast(fp8),
                        rhs=ct["msgp"][:, gl, 0:136],
                        start=(done == 0), stop=(done == tot - 1))
                    done += 1
            rec = pBs.tile([128, 8], fp32, tag="rec")
            nc.vector.reciprocal(out=rec[:], in_=ps_blk[:, 128:136])
            gt = pC.tile([128, 128], fp32, tag="gt")
            nc.vector.tensor_tensor(
                out=gt[:].rearrange("p (h f) -> p h f", f=F),
                in0=ps_blk[:, 0:128].rearrange("p (h f) -> p h f", f=F),
                in1=rec[:].to_broadcast([128, 8, F]), op=Alu.mult)
            if not host["triv_bgat"]:
                nc.vector.tensor_tensor(out=gt[:], in0=gt[:], in1=gl_s["bgat"][:],
                                        op=Alu.add)
            # ---- phase C for tile b ----
            xo = pC.tile([128, 128], fp32, tag="xo")
            nc.sync.dma_start(out=xo[:], in_=xown_d[b * 128:(b + 1) * 128, :])
            t1 = pC.tile([128, 128], fp32, tag="t1")
            nc.vector.tensor_tensor(out=t1[:], in0=xo[:], in1=gt[:], op=Alu.add)

            def layer_norm(tin, g_key, b_key, triv, tagp):
                bst = pBs.tile([128, 6], fp32, tag=f"bst{tagp}")
                nc.vector.bn_stats(out=bst[:], in_=tin[:])
                mv = pBs.tile([128, 2], fp32, tag=f"mv{tagp}")
                nc.vector.bn_aggr(out=mv[:], in_=bst[:])
                # rstd = exp(-0.5 * ln(var + eps)); stays in the exp/ln set
                nc.scalar.activation(out=mv[:, 1:2], in_=mv[:, 1:2],
                                     func=Act.Ln, bias=eps_s[:])
                nc.scalar.activation(out=mv[:, 1:2], in_=mv[:, 1:2],
                                     func=Act.Exp, scale=-0.5)
                o = pC.tile([128, 128], fp32, tag=f"ln{tagp}")
                nc.vector.tensor_scalar(out=o[:], in0=tin[:],
                                        scalar1=mv[:, 0:1], op0=Alu.subtract,
                                        scalar2=mv[:, 1:2], op1=Alu.mult)
                if not triv:
                    nc.vector.tensor_tensor(out=o[:], in0=o[:], in1=gl_s[g_key][:],
                                            op=Alu.mult)
                    nc.vector.tensor_tensor(out=o[:], in0=o[:], in1=gl_s[b_key][:],
                                            op=Alu.add)
                return o

            u = layer_norm(t1, "g1", "b1", host["triv_gb1"], "1")
            u_bf = pC.tile([128, 128], bft, tag="ubf")
            nc.scalar.activation(out=u_bf[:], in_=u[:], func=Act.Copy)
            uT_ps = psC.tile([128, 128], bft, tag="uT")
            nc.tensor.transpose(uT_ps[:], in_=u_bf[:], identity=I128_s[:])
            uT = pC.tile([128, 128], bft, tag="uTs")
            nc.scalar.activation(out=uT[:], in_=uT_ps[:], func=Act.Copy)
            f1ps = psC.tile([128, 2, 128], fp32, tag="f1")
            for j in range(2):
                nc.tensor.matmul(f1ps[:, j, :], lhsT=W1_s[:, j * 128:(j + 1) * 128],
                                 rhs=uT[:], start=True, stop=True)
            r1 = pC.tile([128, 2, 128], bft, tag="r1")
            for j in range(2):
                nc.scalar.activation(out=r1[:, j, :], in_=f1ps[:, j, :],
                                     func=Act.Relu, bias=b1c_s[:, j:j + 1])
            zps = psC.tile([128, 128], fp32, tag="zp")
            for j in range(2):
                nc.tensor.matmul(zps[:], lhsT=r1[:, j, :], rhs=W2_s[:, j, :],
                                 start=(j == 0), stop=(j == 1))
            t2 = pC.tile([128, 128], fp32, tag="t2")
            nc.vector.tensor_tensor(out=t2[:], in0=u[:], in1=zps[:], op=Alu.add)
            if not host["triv_bff2"]:
                nc.vector.tensor_tensor(out=t2[:], in0=t2[:], in1=gl_s["bff2"][:],
                                        op=Alu.add)
            zt = layer_norm(t2, "g2", "b2", host["triv_gb2"], "2")
            nc.sync.dma_start(out=z_d[b * 128:(b + 1) * 128, :], in_=zt[:])

        for p in (psC, pC, psB, pBs, pB):
            p.release()
        cpool.release()

    nc.compile()
    return nc


def kernel(**inputs):
    from concourse.bass_utils import run_bass_kernel_spmd
    import os

    host = _build_host_data(inputs)
    nc = _build_program(host)

    in_maps = []
    for c in range(NCORES):
        m = {
            "xT": host["xT"][c],
            "x_own": host["x_own"][c],
            "Wp": host["Wp"], "I128": host["I128"],
            "W1": host["W1"], "W2": host["W2"], "b1col": host["b1col"],
        }
        if not host["triv_bgat"]:
            m["bgat_r"] = np.tile(host["bias_gat"].reshape(1, -1), (128, 1))
        if not host["triv_bff2"]:
            m["bff2_r"] = np.tile(host["b_ff2"].reshape(1, -1), (128, 1))
        if not host["triv_gb1"]:
            m["g1_r"] = np.tile(host["gamma1"].reshape(1, -1), (128, 1))
            m["b1_r"] = np.tile(host["beta1"].reshape(1, -1), (128, 1))
        if not host["triv_gb2"]:
            m["g2_r"] = np.tile(host["gamma2"].reshape(1, -1), (128, 1))
            m["b2_r"] = np.tile(host["beta2"].reshape(1, -1), (128, 1))
        for s, sname in ((0, "lo"), (1, "hi")):
            m[f"idx_{sname}"] = host["per_core"][c][s]["idx"]
        in_maps.append(m)

    trace = bool(int(os.environ.get("GAT_TRACE", "0")))
    res = run_bass_kernel_spmd(nc, in_maps, core_ids=list(range(NCORES)),
                               trace=trace)
    if trace and res.exec_time_ns:
        print(f"HW exec time: {res.exec_time_ns} ns")
    if bool(int(os.environ.get("GAT_TIME", "0"))):
        try:
            from concourse.timeline_sim import TimelineSim
            ts = TimelineSim(nc)
            dur = ts.simulate()
            print(f"HW exec time: {dur:.0f} ns (cost-model timeline estimate)")
        except Exception as e:
            print("timeline sim failed:", e)

    out = np.zeros((N, D), np.float32)
    for c in range(NCORES):
        on = host["own_nodes"][c]
        mreal = on < N
        out[on[mreal]] = res.results[c]["z"][mreal]
    return out
